# revision 9
# baseline (speedup 1.0000x reference)
"""GCNConv (dense adjacency, 8192 nodes, 512 feat) on 8 Trainium2 NeuronCores.

Math (matches reference):
    A = adj + I
    deg = A.sum(axis=1); dinv = rsqrt(deg)        (deg >= 1 always)
    h = concat(x[:4096] @ Wr, x[4096:] @ Wd)
    out = leaky_relu(dinv[:,None] * (A @ (dinv[:,None] * h)) + bias, 0.01)

Sharding: rows of A / output row-sharded over 8 cores (1024 rows each).

Key structural choices vs the earlier version:
  - g = dinv * h is scaled on the SOURCE core before the AllGather (a core's
    own rows' degree is local), so no deg AllGather / transpose / post-AG
    scaling machinery is needed at all.
  - Hybrid precision main matmul: the first NQ8 DoubleRow fp8 packs
    (2*NQ8 j-blocks) contract fp8-quantized g at 2x PE rate; the remaining
    blocks contract bf16 g exactly. Adjacency values {0,1,2} are exact in
    fp8, so only the quantized g contributes error (~1.6e-2 at NQ8=12).
  - The fp8 DR packs are DMA'd first (t=0) and the first NQ8 stay resident
    in SBUF: they serve both the degree pass and the fp8 main-matmul blocks.
  - The fp8 stationary g8 is produced on receivers by staging the gathered
    g in DR-pack row order and DVE-casting bf16 -> fp8 (round-to-nearest).

Main matmul is computed transposed (out.T = g.T @ A.T), bias fuses into the
LeakyReLU epilogue as a per-partition bias.
"""

import numpy as np
import ml_dtypes

import concourse.bass as bass
import concourse.tile as tile
from concourse import bacc, mybir
from concourse.bass_utils import run_bass_kernel_spmd

N = 8192
C = 512
NCORES = 8
ROWS = N // NCORES       # 1024 rows per core
P = 128
KT = N // P              # 64 contraction blocks of 128 nodes
MT = ROWS // P           # 8 row tiles per core
FT = C // P              # 4 feature tiles for x @ W
CC = C // P              # 4 feature chunks (stationary side of main matmul)
NDR = KT // 2            # 32 fp8 DoubleRow packs (256 j-rows each)
NQ8 = 14                 # DR packs used by the fp8 main-matmul blocks
KT8 = 2 * NQ8            # j-blocks contracted in fp8 (kt 0..KT8-1)
KT16 = KT - KT8          # j-blocks contracted in bf16 (kt KT8..63)
SPK = 4                  # bf16 j-strips per DMA pack
NPK = KT16 // SPK        # bf16 strip packs
NTAILPK = 2              # trailing strip packs with per-cc epilogue overlap
CFP8 = NQ8 // 4          # cores whose g-chunks feed the fp8 blocks fully

F32 = mybir.dt.float32
BF16 = mybir.dt.bfloat16
FP8 = mybir.dt.float8e4


def _emit(nc, tc, dram, io, r, sim_mode=False, parts="all", ag="real"):
    """Emit one full GCN pass. `r` tags pools/tiles for program repetition."""
    adjt_d, adjt8_d, xt_d, w_d, biasc_d, out_d = io

    g_bounce = dram.tile([ROWS, C], BF16, name=f"g_bounce{r}")
    g_full = dram.tile([NCORES, ROWS, C], BF16, addr_space="Shared",
                       name=f"g_full{r}")
    dinv_b = dram.tile([1, ROWS], F32, name=f"dinv_b{r}")

    with tc.tile_pool(name=f"const{r}", bufs=1) as const_pool, \
         tc.tile_pool(name=f"misc{r}", bufs=1) as misc_pool, \
         tc.tile_pool(name=f"pk8r{r}", bufs=1) as pk8r_pool, \
         tc.tile_pool(name=f"hg{r}", bufs=1) as hg_pool:
        ones8_t = const_pool.tile([P, 2, 16], FP8)
        nc.gpsimd.memset(ones8_t[:], 1.0)
        ones1_t = const_pool.tile([1, P], F32)
        nc.gpsimd.memset(ones1_t[:], 1.0)
        bias_pp = const_pool.tile([P, CC], F32)
        nc.sync.dma_start(bias_pp[:],
                          biasc_d.ap().rearrange("(cc p) -> p cc", p=P))
        dinvr_bc = const_pool.tile([P, ROWS], F32)

        # resident fp8 DR packs (deg pass + fp8 main-matmul blocks)
        pk8r_t = pk8r_pool.tile([P, NQ8, 2048], FP8)
        # bf16 stationary for the bf16 main-matmul blocks
        hg_t = hg_pool.tile([P, KT16, C], BF16)
        # fp8 stationary for the fp8 main-matmul blocks
        g8_t = hg_pool.tile([P, NQ8, 2, C], FP8)

        with tc.tile_pool(name=f"xw{r}", bufs=1) as xw_pool, \
             tc.tile_pool(name=f"hps{r}", bufs=2, space="PSUM") as hps_pool, \
             tc.tile_pool(name=f"degps{r}", bufs=1, space="PSUM") as degps_pool, \
             tc.tile_pool(name=f"bcps{r}", bufs=1, space="PSUM") as bcps_pool, \
             tc.tile_pool(name=f"adjs1{r}", bufs=4) as adjs1_pool:
            # ---- DMA front: xt, w first (xW start), then fp8 packs ----
            xt_t = xw_pool.tile([P, FT, ROWS], BF16)
            nc.sync.dma_start(
                xt_t[:], xt_d.ap().rearrange("(f p) i -> p f i", p=P))
            w_t = xw_pool.tile([P, FT, C], BF16)
            nc.sync.dma_start(
                w_t[:], w_d.ap().rearrange("(f p) c -> p f c", p=P))
            for b0 in range(0, NQ8, 4):
                b1 = min(b0 + 4, NQ8)
                nc.sync.dma_start(
                    pk8r_t[:, b0:b1, :],
                    adjt8_d.ap()[b0:b1].rearrange("s p i -> p s i"))

            # ---------------- Phase 1: h_shard = x_shard @ W ----------
            h_sb = xw_pool.tile([P, MT, C], BF16)
            for mt in range(MT):
                h_ps = hps_pool.tile([P, C], F32)
                for ft in range(FT):
                    nc.tensor.matmul(
                        h_ps[:],
                        lhsT=xt_t[:, ft, mt * P:(mt + 1) * P],
                        rhs=w_t[:, ft, :],
                        start=(ft == 0), stop=(ft == FT - 1))
                nc.scalar.copy(h_sb[:, mt, :], h_ps[:])

            # ---------------- Phase 2: deg = row sums of A shard ------
            deg_ps = [degps_pool.tile([1, C], F32, tag=f"degp{i}",
                                      name=f"degp{i}")
                      for i in range(2)]

            def deg_mm(q, pack_ap):
                r3 = pack_ap.rearrange("p (two i) -> p two i", two=2)
                for half in range(2):
                    nc.tensor.matmul(
                        deg_ps[half][:],
                        lhsT=ones8_t[:, :, 0:1],
                        rhs=r3[:, :, half * C:(half + 1) * C],
                        perf_mode=mybir.MatmulPerfMode.DoubleRow,
                        start=(q == 0), stop=(q == NDR - 1))

            for q in range(NQ8):
                deg_mm(q, pk8r_t[:, q, :])
            for dq in range((NDR - NQ8) // 2):     # stream the rest, 2 at a time
                pk8 = adjs1_pool.tile([P, 2, 2048], FP8, tag="pk8")
                nc.sync.dma_start(
                    pk8[:], adjt8_d.ap()[NQ8 + 2 * dq:NQ8 + 2 * dq + 2]
                    .rearrange("s p i -> p s i"))
                for s in range(2):
                    deg_mm(NQ8 + 2 * dq + s, pk8[:, s, :])

            # local dinv of own rows: dinv = rsqrt(deg), straight from PSUM
            rrow = misc_pool.tile([1, ROWS], F32, tag="rrow")
            for half in range(2):
                nc.vector.reciprocal(
                    rrow[:, half * C:(half + 1) * C], deg_ps[half][:])
            drow = misc_pool.tile([1, ROWS], F32, tag="drow")
            nc.scalar.sqrt(drow[:], rrow[:])

            # transposed dinv [P, MT] for scaling h_sb (j on partitions):
            # bounce through DRAM to cross partitions
            nc.sync.dma_start(dinv_b[:], drow[:])
            dinvT = misc_pool.tile([P, MT], F32, tag="dinvT")
            nc.sync.dma_start(
                dinvT[:], dinv_b.rearrange("o (f p) -> (o p) f", p=P))

            # ---- scale own h by dinv (g = dinv * h), bounce, AllGather
            # (bounce in two halves so half 0 overlaps the later scales)
            gb2 = g_bounce.rearrange("(h m p) c -> h p m c", h=2, p=P)
            for hh in range(2):
                for mt in range(4 * hh, 4 * hh + 4):
                    nc.vector.tensor_scalar_mul(
                        h_sb[:, mt, :], h_sb[:, mt, :], dinvT[:, mt:mt + 1])
                nc.sync.dma_start(
                    gb2[hh], h_sb[:, 4 * hh:4 * hh + 4, :])

            if sim_mode or ag == "none":
                nc.sync.dma_start(g_full[0, :, :], g_bounce[:])
            elif ag == "tiny":
                g_tb = dram.tile([8, C], BF16, name=f"g_tb{r}")
                g_tf = dram.tile([NCORES, 8, C], BF16, addr_space="Shared",
                                 name=f"g_tf{r}")
                nc.sync.dma_start(g_tb[:], g_bounce[0:8, :])
                nc.gpsimd.collective_compute(
                    "AllGather", mybir.AluOpType.bypass,
                    replica_groups=[list(range(NCORES))],
                    ins=[g_tb.opt()], outs=[g_tf.opt()])
                nc.sync.dma_start(g_full[0, 0:8, :], g_tf[0, :, :])
            else:
                nc.gpsimd.collective_compute(
                    "AllGather", mybir.AluOpType.bypass,
                    replica_groups=[list(range(NCORES))],
                    ins=[g_bounce.opt()], outs=[g_full.opt()])

            # broadcast over partitions for the epilogue row-scaling
            # (not needed until the epilogue: overlaps the AllGather)
            bc_ps = bcps_pool.tile([P, ROWS], F32)
            for half in range(2):
                nc.tensor.matmul(
                    bc_ps[:, half * C:(half + 1) * C],
                    lhsT=ones1_t[:],
                    rhs=drow[:, half * C:(half + 1) * C],
                    start=True, stop=True)
            nc.vector.tensor_copy(dinvr_bc[:], bc_ps[:])

        if parts == "pre":
            nc.sync.dma_start(out_d.ap()[0:P, 0:MT], dinvT[:])
            return

        # ---------------- Phase 3: load g, quantize fp8 part ----------
        with tc.tile_pool(name=f"adjs2{r}", bufs=5) as adjs2_pool, \
             tc.tile_pool(name=f"stage{r}", bufs=2) as stage_pool, \
             tc.tile_pool(name=f"mmps{r}", bufs=1, space="PSUM") as mmps_pool, \
             tc.tile_pool(name=f"ep{r}", bufs=4) as ep_pool:
            # fp8 cores' chunks: stage in bf16 (DR-pack row order), then
            # DVE-quantize into the fp8 stationary layout [p, q, t, c]
            def cast_chunk(c, nkt):
                st = stage_pool.tile([P, 8, C], BF16, tag="st")
                for s in range(nkt // 2):
                    nc.sync.dma_start(
                        st[:, 2 * s:2 * s + 2, :],
                        g_full[c, 2 * s * P:(2 * s + 2) * P, :].rearrange(
                            "(q t p) c -> p (q t) c", p=P, t=2))
                    nc.vector.tensor_copy(
                        g8_t[:, 4 * c + s, :, :],
                        st[:, 2 * s:2 * s + 2, :])

            def load_chunk16(c, kt0=0):
                nc.sync.dma_start(
                    hg_t[:, 8 * c + kt0 - KT8:8 * c - KT8 + 8, :],
                    g_full[c, kt0 * P:, :].rearrange("(k p) c -> p k c", p=P))

            with tc.high_priority():
                for c in range(CFP8):
                    cast_chunk(c, 8)
                if NQ8 % 4:
                    cast_chunk(CFP8, 2 * (NQ8 % 4))
            if NQ8 % 4:
                load_chunk16(CFP8, kt0=2 * (NQ8 % 4))
            for c in range(CFP8 + 1, NCORES):
                load_chunk16(c)

            # ---------------- Phase 4: main matmul out.T = g.T @ A.T --
            mm_ps = [mmps_pool.tile([P, ROWS], F32, tag=f"mm{cc}",
                                    name=f"mm{cc}")
                     for cc in range(CC)]

            def mm8(q, cc, start):
                r3 = pk8r_t[:, q, :].rearrange("p (two i) -> p two i", two=2)
                for half in range(2):
                    nc.tensor.matmul(
                        mm_ps[cc][:, half * C:(half + 1) * C],
                        lhsT=g8_t[:, q, :, cc * P:(cc + 1) * P],
                        rhs=r3[:, :, half * C:(half + 1) * C],
                        perf_mode=mybir.MatmulPerfMode.DoubleRow,
                        start=start, stop=False)

            def mm16(kt, cc, strip_ap, stop):
                for half in range(2):
                    nc.tensor.matmul(
                        mm_ps[cc][:, half * C:(half + 1) * C],
                        lhsT=hg_t[:, kt - KT8, cc * P:(cc + 1) * P],
                        rhs=strip_ap[:, half * C:(half + 1) * C],
                        start=False, stop=stop)

            for q in range(NQ8):
                for cc in range(CC):
                    mm8(q, cc, start=(q == 0))

            def load_pack(pk):
                t = adjs2_pool.tile([P, SPK, ROWS], BF16, tag="pk2",
                                    name=f"pk2_{pk}")
                nc.sync.dma_start(
                    t[:], adjt_d.ap()[SPK * pk:SPK * (pk + 1)].rearrange(
                        "s p i -> p s i"))
                return t

            for pk in range(NPK - NTAILPK):
                t = load_pack(pk)
                for s in range(SPK):
                    kt = KT8 + SPK * pk + s
                    for cc in range(CC):
                        mm16(kt, cc, t[:, s, :], stop=False)

            # trailing packs: per-cc bursts so each chunk's epilogue
            # overlaps the next chunk's matmuls on the PE
            tail_tiles = [load_pack(pk)
                          for pk in range(NPK - NTAILPK, NPK)]
            ktail0 = KT8 + SPK * (NPK - NTAILPK)
            for cc in range(CC):
                for kt in range(ktail0, KT):
                    ti = (kt - ktail0) // SPK
                    s = (kt - ktail0) % SPK
                    mm16(kt, cc, tail_tiles[ti][:, s, :],
                         stop=(kt == KT - 1))
                for eh in range(2):
                    sl = slice(eh * C, (eh + 1) * C)
                    t1 = ep_pool.tile([P, C], F32, tag="t1")
                    nc.vector.tensor_mul(t1[:], mm_ps[cc][:, sl],
                                         dinvr_bc[:, sl])
                    t2 = ep_pool.tile([P, C], F32, tag="t2")
                    nc.scalar.activation(
                        t2[:], t1[:], mybir.ActivationFunctionType.Lrelu,
                        bias=bias_pp[:, cc:cc + 1], alpha=0.01)
                    nc.sync.dma_start(
                        out_d.ap()[cc * P:(cc + 1) * P, sl], t2[:])


def build_kernel(reps: int = 1, sim_mode: bool = False, parts: str = "all",
                 ag: str = "real"):
    """Build and compile the SPMD Bass program (identical on all 8 cores)."""
    nc = bacc.Bacc("TRN2", target_bir_lowering=False, debug=False,
                   num_devices=NCORES)

    adjt_d = nc.dram_tensor("adjt", [KT16, P, ROWS], BF16,
                            kind="ExternalInput")
    adjt8_d = nc.dram_tensor("adjt8", [NDR, P, 2048], FP8,
                             kind="ExternalInput")
    xt_d = nc.dram_tensor("xt", [C, ROWS], BF16, kind="ExternalInput")
    w_d = nc.dram_tensor("w", [C, C], BF16, kind="ExternalInput")
    biasc_d = nc.dram_tensor("biasc", [C], F32, kind="ExternalInput")
    out_d = nc.dram_tensor("out", [C, ROWS], F32, kind="ExternalOutput")
    io = (adjt_d, adjt8_d, xt_d, w_d, biasc_d, out_d)

    with tile.TileContext(nc) as tc:
        with tc.tile_pool(name="dram", bufs=1, space="DRAM") as dram:
            if reps == 0:
                with tc.tile_pool(name="nullp", bufs=1) as np_pool:
                    z = np_pool.tile([P, CC], F32)
                    nc.sync.dma_start(
                        z[:], biasc_d.ap().rearrange("(cc p) -> p cc", p=P))
            for r in range(reps):
                _emit(nc, tc, dram, io, r, sim_mode=sim_mode, parts=parts,
                      ag=ag)

    nc.compile()
    return nc


def prepare_inputs(x, adj, weightr, weightd, bias):
    """Host-side sharding/layout. Returns in_maps for the 8 cores."""
    x = np.asarray(x, dtype=np.float32)
    adj = np.asarray(adj, dtype=np.float32)
    weightr = np.asarray(weightr, dtype=np.float32)
    weightd = np.asarray(weightd, dtype=np.float32)
    bias = np.ascontiguousarray(np.asarray(bias, dtype=np.float32))

    wr16 = weightr.astype(ml_dtypes.bfloat16)
    wd16 = weightd.astype(ml_dtypes.bfloat16)
    idx = np.arange(ROWS)
    # A values are only 0/1/2: build uint8 once, then LUT-cast (fast + exact)
    lut16 = np.array([0x0000, 0x3F80, 0x4000], dtype=np.uint16)  # bf16 bits
    lut8 = np.array([0x00, 0x38, 0x40], dtype=np.uint8)          # e4m3 bits

    in_maps = []
    for c in range(NCORES):
        rows = slice(c * ROWS, (c + 1) * ROWS)
        ai = adj[rows, :].T.astype(np.uint8)             # [N, ROWS] 0/1
        ai[c * ROWS + idx, idx] += 1                     # fold in self-loop
        # bf16 strips only for the bf16-contracted blocks kt KT8..63
        adjt = (lut16[ai[KT8 * P:, :]].view(ml_dtypes.bfloat16)
                .reshape(KT16, P, ROWS))
        # DoubleRow packs [32, 128, 2048]: row p = [j=256q+p | j=256q+128+p]
        adjt8 = np.ascontiguousarray(
            lut8[ai].view(ml_dtypes.float8_e4m3)
            .reshape(NDR, 2, P, ROWS).transpose(0, 2, 1, 3)
        ).reshape(NDR, P, 2048)
        xt = np.ascontiguousarray(x[rows, :].T).astype(ml_dtypes.bfloat16)
        w = wr16 if c < NCORES // 2 else wd16
        in_maps.append({"adjt": adjt, "adjt8": adjt8, "xt": xt, "w": w,
                        "biasc": bias})
    return in_maps


_NC_CACHE = {}


def kernel(x, adj, weightr, weightd, bias):
    if "nc" not in _NC_CACHE:
        _NC_CACHE["nc"] = build_kernel(reps=1)
    nc = _NC_CACHE["nc"]
    in_maps = prepare_inputs(x, adj, weightr, weightd, bias)
    res = run_bass_kernel_spmd(nc, in_maps, list(range(NCORES)))
    out = np.concatenate(
        [np.ascontiguousarray(res.results[c]["out"].T) for c in range(NCORES)],
        axis=0)
    return out
